# revision 85
# baseline (speedup 1.0000x reference)
"""GraphSAGE 3-layer + output projection on 8 Trainium2 NeuronCores.

Sharding: nodes (and dst-partitioned edges) split across 8 cores, 1280
nodes/core (N padded 10000->10240). Per layer: all cores hold the full
previous-layer activations in DRAM (x0 stored in the AG-remap order so
one index table serves all layers); each core gathers its edges' unique
source rows with batched dma_gather ops (the dma_gather stream is the
wall at ~8.4ns/row — SWDGE descriptor generation + SDMA packet
processing — so rows are deduped per dst tile and pad slots are skipped
via trailing -1 indices), segment-sums them on the TensorEngine via
host-built count-matrix matmuls (bf16, free dim 512), scales by 1/deg,
transposes to feature-major, and applies lin_l/lin_r as bf16 matmuls.
Gather dedup runs over 256-dst PAIRS of tiles (PD=2): ~10% fewer rows
and half the gather instructions vs per-tile, at the cost of one count
matmul per dst tile per gathered subtile (Tensor has slack). Gathers
and their count matmuls are emitted SEPARATELY so the Pool queue gets
gathers early while the Tensor queue gets each dense group's matmuls
before later pairs' bulk count-matmul work (the AG-trigger input chain
was otherwise queued behind it). Each pair's gather splits into op1
(sources in AG pieces A1+A2, i.e. remap < 4096) and op2 (the rest):
op1 starts as soon as piece A2 lands — 25-50us before B — and op2's
piece-C gate is absorbed by op1's runtime (op1 work ~55us/layer matches
the A2->C landing window; gating op1 on A1 alone would leave a ~77us
hole). Layer 0 gathers from local x0 with no AG gates, so it uses a
per-pair interleaved order that fires the first AG piece ~60us earlier.
pa bufs=4 keeps 4 tiles' PSUM accumulations open across ops.
The AG runs in 4 pieces (tiles 0-1 / 2-3 / 4-7 / 8-9): the first piece
covers only pair 0's tiles so its trigger depends on just op2m(0) —
this fires the AG chain ~25us earlier per boundary than a 4-tile first
piece (the Tile scheduler orders triggers by modeled dep-times, and its
hardcoded collective cost model (15us + size/40GBps) overestimates AG
durations ~3x, pushing C-dependent work late in the queue; an earlier-
dependency first piece is the workaround that sticks).

Measured (profiled on HW, run variance +-3%): original baseline 767us
span with Pool 543us busy / Tensor 295us. Final: ~665-680us (mean
~674us) with Pool ~450us / Tensor ~325us. Remaining idle ~260us: AG
pieces are SKEW-dominated (a 2MB piece measures 17-51us; the slowest
core gates each piece; pieces serialize on one CC stream), so layer
boundaries stall ~60-90us waiting for op1's A+B gate. Rejected by
measurement: dense-adjacency aggregation on the TensorEngine for 2
tiles (+56us: PE runs at the HAM-throttled 1.2GHz clock since tensor
work is bursty — matmul pairs ~300ns vs 131ns warm — and the
10MB/layer x-block stream contends on DMA queues); splitting op1 by AG
piece A/B (+60us: 30 extra ~1us-fixed-cost gather instructions);
4-tile merged gathers (+220us: overlap collapse; the stream is also
near the SDMA packet-processing limit); 5 AG pieces (+35us: extra
stream op + dense-group fragmentation beats the finer pipelining);
single_packet=True (runtime crash); scheduler SWDGE-model inflation
(no effect); emitting pair-4 op1 matmuls early on the last layer to
trim the tail (tail grew instead); xs1/oh1 bufs 2->3 (+120us: extra
slack lets the scheduler reorder badly); op2g(0) pulled to 2nd Pool
slot for an earlier first trigger (+15us: the ag(0) head-of-line stall
costs more than the earlier trigger buys); 3-way op1 split by piece
A1/A2 to fill the A1->A2 landing window (+30us: ten extra gather
instructions plus buffer slack outweigh the fill). The emission order
here is a measured local optimum — perturbations regress.
Next levers if revisited: reduce cross-core AG skew
(stagger-free triggers), a second CC stream, or a gather mechanism
cheaper than ~8.4ns/row.
"""
import sys, types, ctypes, contextlib

import numpy as np


def _install_ntff_hook():
    # antenv.axon_hooks is missing in this image; provide it so
    # bass_utils trace=True can profile via libaxon_pjrt.so.
    if "antenv.axon_hooks" in sys.modules:
        return
    try:
        import antenv  # noqa: F401
    except ImportError:
        return
    mod = types.ModuleType("antenv.axon_hooks")
    state = {"hook": None}
    mod.set_axon_ntff_profile_hook = lambda h: state.__setitem__("hook", h)
    mod.get_axon_ntff_profile_hook = lambda: state["hook"]
    sys.modules["antenv.axon_hooks"] = mod
    try:
        lib = ctypes.CDLL('/opt/axon/libaxon_pjrt.so')
    except OSError:
        return
    if not hasattr(lib, "axon_start_nrt_profile"):
        return
    lib.axon_start_nrt_profile.argtypes = [ctypes.POINTER(ctypes.c_int64), ctypes.c_size_t]
    lib.axon_start_nrt_profile.restype = ctypes.c_int64
    lib.axon_stop_nrt_profile.argtypes = [ctypes.c_char_p]
    lib.axon_stop_nrt_profile.restype = ctypes.c_int64

    @contextlib.contextmanager
    def _hook(output_dir, device_ids):
        import jax
        jax.devices()
        if device_ids:
            ids = (ctypes.c_int64 * len(device_ids))(*device_ids)
            rc = lib.axon_start_nrt_profile(ids, len(device_ids))
        else:
            rc = lib.axon_start_nrt_profile(None, 0)
        if rc != 0:
            raise RuntimeError(f"axon_start_nrt_profile rc={rc}")
        try:
            yield
        finally:
            n = lib.axon_stop_nrt_profile(str(output_dir).encode())
            print(f"profile: {n} file(s) written to {output_dir}", file=sys.stderr)

    state["hook"] = _hook


_install_ntff_hook()

import concourse.bass2jax as _b2j
_orig_cc_hook = _b2j.neuronx_cc_hook
def _dbg_cc_hook(*a, **kw):
    try:
        return _orig_cc_hook(*a, **kw)
    except BaseException:
        import traceback
        traceback.print_exc()
        raise
_b2j.neuronx_cc_hook = _dbg_cc_hook

import concourse.hw_specs as _hw_specs
# calibrate the tile scheduler's SWDGE model to the measured dma_gather
# descriptor-generation rate (~8 ns/idx on HW; set higher so the scheduler
# slots AG triggers after fewer gathers — measured triggers otherwise fire
# ~60us after their deps are ready)
_hw_specs.TRN2Spec.SWDGE_NS_PER_DESCRIPTOR = 8.0

import concourse.bass as bass
import concourse.tile as tile
from concourse import mybir, bacc
from concourse.bass_utils import run_bass_kernel_spmd

F32 = mybir.dt.float32
F32R = mybir.dt.float32r
BF16 = mybir.dt.bfloat16
I32 = mybir.dt.int32
I16 = mybir.dt.int16

N, D, H, O = 10000, 512, 512, 128
C = 8              # cores
NP = 10240         # padded node count
NCORE = NP // C    # 1280 nodes per core
NT = NCORE // 128  # 10 dst tiles per core
GROUPS = [(0, 4), (4, 8), (8, 10)]  # dense/AG pieces by dst tile range
PD = 2             # dst tiles per gather-dedup pair
NPAIR = NT // PD   # 5 dedup pairs per core
AB_ROWS = 4096     # xg rows gating op1 (AG pieces A1+A2); op1 starts at
                   # A2-land, ~25-50us before piece B lands; op2 covers the
                   # rest and its C gate is absorbed by op1's runtime


def _host_prep(x, edge_index):
    src = np.asarray(edge_index[0], dtype=np.int64)
    dst = np.asarray(edge_index[1], dtype=np.int64)
    deg = np.bincount(dst, minlength=NP).astype(np.float64)
    invdeg = (1.0 / np.maximum(deg, 1.0)).astype(np.float32)

    # piece-wise AllGather layout for layers 1,2: an AG piece covering R
    # local rows per core lays its output out as piece_base + c*R + off.
    # Pieces: tiles 0-3 (512 rows), 4-7 (512), 8-9 (256); op1 gates on the
    # FIRST piece only (one 4MB transfer, no intra-A serialization).
    # x0 is stored in the same remapped order so the same gather indices
    # serve all three layers.
    allnodes = np.arange(NP, dtype=np.int64)
    cc, loc = allnodes // NCORE, allnodes % NCORE
    remap = np.where(
        loc < 512, cc * 512 + loc,
        np.where(loc < 1024, 4096 + cc * 512 + (loc - 512),
                 8192 + cc * 256 + (loc - 1024))).astype(np.int64)

    # per (core, 256-dst dedup pair): unique sources split into op1
    # (remap < AB_ROWS) and op2 (remap >= AB_ROWS); deduping over pairs of
    # dst tiles cuts gathered rows ~10% vs per-tile (each subtile then feeds
    # two count matmuls, one per dst tile). oh[slot, sub, dstoff256] = count
    uniq = [[None] * NPAIR for _ in range(C)]
    n1 = np.zeros((C, NPAIR), np.int64)
    n2 = np.zeros((C, NPAIR), np.int64)
    for c in range(C):
        for p in range(NPAIR):
            g0 = (c * NT + p * PD) * 128
            sel = (dst >= g0) & (dst < g0 + PD * 128)
            s_e = src[sel]
            d_e = (dst[sel] - g0).astype(np.int64)
            us = np.unique(s_e)
            m1 = remap[us] < AB_ROWS
            us1, us2 = us[m1], us[~m1]
            uniq[c][p] = (us1, us2, s_e, d_e)
            n1[c, p], n2[c, p] = len(us1), len(us2)

    cnt1m = n1.max(axis=0)
    cnt2m = n2.max(axis=0)
    T1 = np.maximum(np.ceil(cnt1m / 128).astype(np.int64), 1)
    T2 = np.maximum(np.ceil(cnt2m / 128).astype(np.int64), 1)

    # one dma_gather per (pair, op); segment layout [op1_p | op2_p] per pair
    bases1 = np.zeros(NPAIR, np.int64)
    bases2 = np.zeros(NPAIR, np.int64)
    pos = 0
    for p in range(NPAIR):
        bases1[p] = pos
        pos += int(T1[p])
        bases2[p] = pos
        pos += int(T2[p])
    ST = pos

    gidx12 = np.full((C, 128, ST * 8), -1, np.int16)   # xg/x0r rows
    ohv = np.zeros((C, 128, ST, PD * 128), np.float32)  # [slot, sub, dstoff]

    def fill(tbl, vals, col0):
        # value i lands at gather position col0*128 + i; idx table wraps
        # position j at [j % 16, j // 16], replicated over 8 row groups.
        if len(vals) == 0:
            return
        i = np.arange(len(vals))
        pos = col0 * 128 + i
        for r in range(8):
            tbl[16 * r + pos % 16, pos // 16] = vals

    for c in range(C):
        for p in range(NPAIR):
            us1, us2, s_e, d_e = uniq[c][p]
            slot_of = {}
            for us_i, cm, b in ((us1, cnt1m[p], bases1[p]),
                                (us2, cnt2m[p], bases2[p])):
                # pad to the cross-core max with idx 0 (the -1 tail after
                # that is skipped via num_idxs_reg)
                pp = np.zeros(int(cm), np.int64)
                pp[:len(us_i)] = us_i
                fill(gidx12[c], remap[pp].astype(np.int16), int(b))
                for i, s in enumerate(us_i):
                    slot_of[int(s)] = (int(b) + i // 128, i % 128)
            for s_i, d_i in zip(s_e, d_e):
                sub, pslot = slot_of[int(s_i)]
                ohv[c, pslot, sub, d_i] += 1.0

    x_pad = np.zeros((NP, D), np.float32)
    x_pad[:N] = np.asarray(x, dtype=np.float32)
    x0r = np.zeros((NP, D), np.float32)
    x0r[remap] = x_pad

    invdeg_sb = np.empty((C, 128, NT), np.float32)
    for c in range(C):
        invdeg_sb[c] = invdeg[c * NCORE:(c + 1) * NCORE].reshape(NT, 128).T

    xT0 = np.empty((C, 128, 4, NCORE), np.float32)
    for c in range(C):
        xT0[c] = x_pad[c * NCORE:(c + 1) * NCORE].reshape(NCORE, 4, 128).transpose(2, 1, 0)

    import ml_dtypes
    ohv = ohv.astype(ml_dtypes.bfloat16)
    return (x0r, gidx12, ohv, invdeg_sb, xT0,
            T1, T2, bases1, bases2, cnt1m, cnt2m, ST)


def _wsb(w):
    # [K, M] -> SBUF layout [128, K/128, M], bf16
    import ml_dtypes
    w = np.asarray(w, np.float32)
    return np.ascontiguousarray(
        w.reshape(w.shape[0] // 128, 128, w.shape[1]).transpose(1, 0, 2)
    ).astype(ml_dtypes.bfloat16)


def _bsb(b):
    # [M] -> [128, M/128]
    b = np.asarray(b, np.float32)
    return np.ascontiguousarray(b.reshape(b.shape[0] // 128, 128).T)


def _build_program(T1, T2, bases1, bases2, cnt1m, cnt2m, ST):
    NE1MAX, NE2MAX = int(T1.max()), int(T2.max())
    nc = bacc.Bacc(None, target_bir_lowering=False, debug=False, num_devices=C,
                   dynamic_dma_scratch_size=16384)

    x0_d = nc.declare_dram_parameter("x_full0", [NP, D], BF16, isOutput=False)
    gidx12_d = nc.declare_dram_parameter("gidx12", [128, ST * 8], I16, isOutput=False)
    oh_d = nc.declare_dram_parameter("ohv", [128, ST, PD * 128], BF16, isOutput=False)
    invdeg_d = nc.declare_dram_parameter("invdeg", [128, NT], F32, isOutput=False)
    ident_d = nc.declare_dram_parameter("ident", [128, 128], F32, isOutput=False)
    identb_d = nc.declare_dram_parameter("identb", [128, 128], BF16, isOutput=False)
    zeros_d = nc.declare_dram_parameter("zeros", [128, 1], F32, isOutput=False)
    xT0_d = nc.declare_dram_parameter("xT0", [128, 4, NCORE], BF16, isOutput=False)
    w_d = {}
    for l in range(3):
        w_d[f"wl{l}"] = nc.declare_dram_parameter(f"wl{l}", [128, 4, H], BF16, isOutput=False)
        w_d[f"wr{l}"] = nc.declare_dram_parameter(f"wr{l}", [128, 4, H], BF16, isOutput=False)
        w_d[f"b{l}"] = nc.declare_dram_parameter(f"b{l}", [128, 4], F32, isOutput=False)
    wout_d = nc.declare_dram_parameter("wout", [128, 4, O], BF16, isOutput=False)
    bout_d = nc.declare_dram_parameter("bout", [128, 1], F32, isOutput=False)
    out_d = nc.declare_dram_parameter("out", [NCORE, O], F32, isOutput=True)

    xg = [None, nc.dram_tensor("xg1", [NP, D], BF16, addr_space="Shared"),
          nc.dram_tensor("xg2", [NP, D], BF16, addr_space="Shared")]
    xc = [None, nc.dram_tensor("xc1", [NCORE, D], BF16),
          nc.dram_tensor("xc2", [NCORE, D], BF16)]

    with tile.TileContext(nc) as tc:
        with tc.tile_pool(name="const", bufs=1) as constp, \
             tc.tile_pool(name="xT", bufs=2) as xTp, \
             tc.tile_pool(name="aggT", bufs=1) as aggTp, \
             tc.tile_pool(name="xs1", bufs=2) as xs1p, \
             tc.tile_pool(name="xs2", bufs=2) as xs2p, \
             tc.tile_pool(name="oh1", bufs=2) as oh1p, \
             tc.tile_pool(name="oh2", bufs=2) as oh2p, \
             tc.tile_pool(name="agg", bufs=2) as aggp, \
             tc.tile_pool(name="xnm", bufs=2) as xnmp, \
             tc.tile_pool(name="wts", bufs=2) as wp, \
             tc.tile_pool(name="dmy", bufs=3) as dmyp, \
             tc.tile_pool(name="pa", bufs=4, space="PSUM") as pap, \
             tc.tile_pool(name="pt", bufs=1, space="PSUM") as ptp, \
             tc.tile_pool(name="pd", bufs=2, space="PSUM") as pdp:

            # ---- load constants ----
            gidx12_sb = constp.tile([128, ST * 8], I16)
            nc.sync.dma_start(gidx12_sb[:], gidx12_d[:])
            invdeg_sb = constp.tile([128, NT], F32)
            nc.sync.dma_start(invdeg_sb[:], invdeg_d[:])
            ident = constp.tile([128, 128], F32)
            nc.sync.dma_start(ident[:], ident_d[:])
            identb = constp.tile([128, 128], BF16)
            nc.sync.dma_start(identb[:], identb_d[:])
            wsb = {}
            for l in range(3):
                wsb[f"b{l}"] = constp.tile([128, 4], F32, name=f"bsb{l}")
                nc.sync.dma_start(wsb[f"b{l}"][:], w_d[f"b{l}"][:])
            wout_sb = constp.tile([128, 4, O], BF16)
            nc.sync.dma_start(wout_sb[:], wout_d[:])
            bout_sb = constp.tile([128, 1], F32)
            nc.sync.dma_start(bout_sb[:], bout_d[:])

            xT_cur = xTp.tile([128, 4, NCORE], BF16)
            nc.sync.dma_start(xT_cur[:], xT0_d[:])

            gate_in = None
            for l in range(3):
                gidx = gidx12_sb
                aggT = aggTp.tile([128, 4, NCORE], BF16)
                xT_next = xTp.tile([128, 4, NCORE], BF16)
                wlr = wp.tile([128, 8, H], BF16, name="wlr")
                nc.sync.dma_start(wlr[:, 0:4, :], w_d[f"wl{l}"][:])
                nc.sync.dma_start(wlr[:, 4:8, :], w_d[f"wr{l}"][:])
                wl, wr, bb = wlr[:, 0:4, :], wlr[:, 4:8, :], wsb[f"b{l}"]

                pa_of = {}

                xs1_of, xs2_of = {}, {}

                def do_op1g(p):
                    # gather the pair's op1 srcs (AG pieces A+B); the count
                    # matmuls are emitted separately (do_op1m) so Tensor-queue
                    # order can differ from Pool-queue order.
                    ne = int(T1[p])
                    b0 = int(bases1[p])
                    xs = xs1p.tile([128, NE1MAX, D], BF16, name="xs1")
                    nc.vector.memset(xs[:, ne - 1, :], 0.0)
                    src_ap = x0_d[0:AB_ROWS, :] if l == 0 else xg[l][0:AB_ROWS, :]
                    nc.gpsimd.dma_gather(
                        out_ap=xs[:, :ne, :], in_ap=src_ap,
                        idxs_ap=gidx[:, b0 * 8:(b0 + ne) * 8],
                        num_idxs=ne * 128, num_idxs_reg=int(cnt1m[p]),
                        elem_size=D, single_packet=False)
                    oh = oh1p.tile([128, NE1MAX, PD * 128], BF16, name="oh1")
                    nc.sync.dma_start(oh[:, :ne, :], oh_d[:, b0:b0 + ne, :])
                    xs1_of[p] = (xs, oh)

                def do_op1m(p):
                    # count matmuls for the pair's op1 subtiles; opens both
                    # tiles' PSUM accumulations (one matmul per dst tile per
                    # gathered subtile).
                    ne = int(T1[p])
                    xs, oh = xs1_of.pop(p)
                    for j in range(PD):
                        pa = pap.tile([128, D], F32, name="pa")
                        pa_of[p * PD + j] = pa
                        for e in range(ne):
                            nc.tensor.matmul(
                                pa[:], lhsT=oh[:, e, j * 128:(j + 1) * 128],
                                rhs=xs[:, e, :],
                                start=(e == 0), stop=False)

                def do_op2g(p):
                    # gather the pair's op2 srcs (AG piece C)
                    ne = int(T2[p])
                    b0 = int(bases2[p])
                    xs = xs2p.tile([128, NE2MAX, D], BF16, name="xs2")
                    nc.vector.memset(xs[:, ne - 1, :], 0.0)
                    src_ap = x0_d[:] if l == 0 else xg[l][:]
                    nc.gpsimd.dma_gather(
                        out_ap=xs[:, :ne, :], in_ap=src_ap,
                        idxs_ap=gidx[:, b0 * 8:(b0 + ne) * 8],
                        num_idxs=ne * 128, num_idxs_reg=int(cnt2m[p]),
                        elem_size=D, single_packet=False)
                    oh = oh2p.tile([128, NE2MAX, PD * 128], BF16, name="oh2")
                    nc.sync.dma_start(oh[:, :ne, :], oh_d[:, b0:b0 + ne, :])
                    xs2_of[p] = (xs, oh)

                def do_op2m(p):
                    # close both accumulations, scale by 1/deg, transpose
                    # to feat-major.
                    ne = int(T2[p])
                    xs, oh = xs2_of.pop(p)
                    for j in range(PD):
                        t = p * PD + j
                        pa = pa_of.pop(t)
                        for e in range(ne):
                            nc.tensor.matmul(
                                pa[:], lhsT=oh[:, e, j * 128:(j + 1) * 128],
                                rhs=xs[:, e, :],
                                start=False, stop=(e == ne - 1))
                        agg = aggp.tile([128, D], F32, name="agg")
                        nc.scalar.activation(
                            agg[:], pa[:], mybir.ActivationFunctionType.Copy,
                            scale=invdeg_sb[:, t:t + 1])
                        for k in range(4):
                            pt = ptp.tile([128, 128], F32, name="pt")
                            nc.tensor.transpose(pt[:], agg[:, k * 128:(k + 1) * 128], ident[:])
                            nc.vector.tensor_copy(aggT[:, k, t * 128:(t + 1) * 128], pt[:])

                def do_dense_group(goff, gsz):
                    for m in range(4):
                        pd = pdp.tile([128, 512], F32, name="pd")
                        for k in range(4):
                            nc.tensor.matmul(
                                pd[:, :gsz],
                                lhsT=wl[:, k, m * 128:(m + 1) * 128],
                                rhs=aggT[:, k, goff:goff + gsz],
                                start=(k == 0), stop=False)
                        for k in range(4):
                            nc.tensor.matmul(
                                pd[:, :gsz],
                                lhsT=wr[:, k, m * 128:(m + 1) * 128],
                                rhs=xT_cur[:, k, goff:goff + gsz],
                                start=False, stop=(k == 3))
                        nc.scalar.activation(
                            xT_next[:, m, goff:goff + gsz], pd[:, :gsz],
                            mybir.ActivationFunctionType.Relu,
                            bias=bb[:, m:m + 1])
                    if l < 2:
                        for t in range(goff // 128, (goff + gsz) // 128):
                            xnm = xnmp.tile([128, D], BF16, name="xnm")
                            for k in range(4):
                                pt = ptp.tile([128, 128], BF16, name="ptx")
                                nc.tensor.transpose(
                                    pt[:], xT_next[:, k, t * 128:(t + 1) * 128],
                                    identb[:])
                                nc.vector.tensor_copy(xnm[:, k * 128:(k + 1) * 128], pt[:])
                            nc.sync.dma_start(xc[l + 1][t * 128:(t + 1) * 128, :], xnm[:])

                def do_final_group(t0, t1, xT_src):
                    goff, gsz = t0 * 128, (t1 - t0) * 128
                    pd = pdp.tile([128, 512], F32, name="pd")
                    for k in range(4):
                        nc.tensor.matmul(
                            pd[:, :gsz],
                            lhsT=wout_sb[:, k, :],
                            rhs=xT_src[:, k, goff:goff + gsz],
                            start=(k == 0), stop=(k == 3))
                    oT = aggp.tile([128, 512], F32)
                    nc.scalar.activation(
                        oT[:, :gsz], pd[:, :gsz],
                        mybir.ActivationFunctionType.Identity, bias=bout_sb[:, 0:1])
                    for tt in range(gsz // 128):
                        t = t0 + tt
                        pt = ptp.tile([128, 128], F32)
                        nc.tensor.transpose(pt[:], oT[:, tt * 128:(tt + 1) * 128], ident[:])
                        onm = xnmp.tile([128, O], F32)
                        nc.vector.tensor_copy(onm[:], pt[:])
                        nc.sync.dma_start(out_d[t * 128:(t + 1) * 128, :], onm[:])

                # Pool-queue order: each AG trigger is emitted only after the
                # NEXT group's op1 gathers, so a trigger blocked on its dense
                # group can never starve the gather stream (no head-of-line
                # cycle), while still firing ~2 groups earlier than end-of-layer.
                ag_bounds = [(0, 4096), (4096, 8192), (8192, 10240)]

                def do_ag(gi):
                    t0, t1 = GROUPS[gi]
                    with tc.high_priority():
                        do_dense_group(t0 * 128, (t1 - t0) * 128)
                        if l < 2:
                            lo, hi = ag_bounds[gi]
                            nc.gpsimd.collective_compute(
                                "AllGather", mybir.AluOpType.bypass,
                                replica_groups=[list(range(C))],
                                ins=[xc[l + 1][t0 * 128:t1 * 128, :]],
                                outs=[xg[l + 1][lo:hi, :]])
                            return None
                        do_final_group(t0, t1, xT_next)
                    return None

                # Pool-queue order: each AG trigger is emitted after the next
                # op1 pair past the gathers its dense group depends on, so a
                # trigger blocked on its dense group never starves the gather
                # stream, while still firing well before end-of-layer.
                # Pool gets gathers as early as their AG gates allow; Tensor
                # gets each dense piece's matmuls BEFORE later pairs' count
                # matmuls, so the AG-trigger input chain is never queued
                # behind bulk count-matmul work. The first AG piece covers
                # just pair 0's tiles, so its trigger needs only op2m(0).
                if l == 0:
                    # layer 0 gathers from local x0 — no AG gates at all, so
                    # interleave ops per pair and fire each AG piece as soon
                    # as its tiles' aggregations close
                    for p in range(NPAIR):
                        do_op1g(p)
                        do_op2g(p)
                        do_op1m(p)
                        do_op2m(p)
                        if p == 1:
                            do_ag(0)
                        elif p == 3:
                            do_ag(1)
                        elif p == 4:
                            do_ag(2)
                else:
                    # op1 stream (A-gated) first; op2 stream (C-gated) after,
                    # with each piece's trigger right after its last op2m
                    do_op1g(0)
                    do_op1g(1)
                    do_op1g(2)
                    do_op1g(3)
                    do_op1g(4)
                    do_op1m(0)
                    do_op2g(0)
                    do_op2m(0)
                    do_op1m(1)
                    do_op2g(1)
                    do_op2m(1)
                    do_ag(0)
                    do_op1m(2)
                    do_op2g(2)
                    do_op2m(2)
                    do_op1m(3)
                    do_op2g(3)
                    do_op2m(3)
                    do_ag(1)
                    do_op1m(4)
                    do_op2g(4)
                    do_op2m(4)
                    do_ag(2)
                xT_cur = xT_next

    nc.compile()
    return nc


def _run(inputs, trace=False):
    x = inputs["x"]
    edge_index = inputs["edge_index"]
    (x0r, gidx12, ohv, invdeg_sb, xT0,
     T1, T2, bases1, bases2, cnt1m, cnt2m, ST) = _host_prep(x, edge_index)
    nc = _build_program(T1, T2, bases1, bases2, cnt1m, cnt2m, ST)

    import ml_dtypes
    shared = {
        "x_full0": x0r.astype(ml_dtypes.bfloat16),
        "wout": _wsb(inputs["w_out"]),
        "bout": np.asarray(inputs["b_out"], np.float32).reshape(128, 1),
        "ident": np.eye(128, dtype=np.float32),
        "identb": np.eye(128, dtype=np.float32).astype(ml_dtypes.bfloat16),
        "zeros": np.zeros((128, 1), np.float32),
    }
    for l in range(3):
        shared[f"wl{l}"] = _wsb(inputs[f"w_l{l}"])
        shared[f"wr{l}"] = _wsb(inputs[f"w_r{l}"])
        shared[f"b{l}"] = _bsb(inputs[f"b_l{l}"])

    in_maps = []
    for c in range(C):
        m = dict(shared)
        m["gidx12"] = np.ascontiguousarray(gidx12[c])
        m["ohv"] = np.ascontiguousarray(ohv[c])
        m["invdeg"] = np.ascontiguousarray(invdeg_sb[c])
        m["xT0"] = np.ascontiguousarray(xT0[c]).astype(ml_dtypes.bfloat16)
        in_maps.append(m)

    res = run_bass_kernel_spmd(nc, in_maps, list(range(C)), trace=trace)
    out = np.concatenate([res.results[c]["out"] for c in range(C)], axis=0)[:N]
    return out.astype(np.float32), res


def kernel(**inputs):
    out, _ = _run(inputs, trace=False)
    return out


def kernel_timed(**inputs):
    out, res = _run(inputs, trace=True)
    return out, res



# revision 86
# speedup vs baseline: 1.0194x; 1.0194x over previous
"""GraphSAGE 3-layer + output projection on 8 Trainium2 NeuronCores.

Sharding: nodes (and dst-partitioned edges) split across 8 cores, 1280
nodes/core (N padded 10000->10240). Per layer: all cores hold the full
previous-layer activations in DRAM (x0 stored in the AG-remap order so
one index table serves all layers); each core gathers its edges' unique
source rows with batched dma_gather ops (the dma_gather stream is the
wall at ~8.4ns/row — SWDGE descriptor generation + SDMA packet
processing — so rows are deduped per dst tile and pad slots are skipped
via trailing -1 indices), segment-sums them on the TensorEngine via
host-built count-matrix matmuls (bf16, free dim 512), scales by 1/deg,
transposes to feature-major, and applies lin_l/lin_r as bf16 matmuls.
Gather dedup runs over 256-dst PAIRS of tiles (PD=2): ~10% fewer rows
and half the gather instructions vs per-tile, at the cost of one count
matmul per dst tile per gathered subtile (Tensor has slack). Gathers
and their count matmuls are emitted SEPARATELY so the Pool queue gets
gathers early while the Tensor queue gets each dense group's matmuls
before later pairs' bulk count-matmul work (the AG-trigger input chain
was otherwise queued behind it). Each pair's gather splits into op1
(sources in AG pieces A1+A2, i.e. remap < 4096) and op2 (the rest):
op1 starts as soon as piece A2 lands — 25-50us before B — and op2's
piece-C gate is absorbed by op1's runtime (op1 work ~55us/layer matches
the A2->C landing window; gating op1 on A1 alone would leave a ~77us
hole). Layer 0 gathers from local x0 with no AG gates, so it uses a
per-pair interleaved order that fires the first AG piece ~60us earlier.
pa bufs=4 keeps 4 tiles' PSUM accumulations open across ops.
The AG runs in 4 pieces (tiles 0-1 / 2-3 / 4-7 / 8-9): the first piece
covers only pair 0's tiles so its trigger depends on just op2m(0) —
this fires the AG chain ~25us earlier per boundary than a 4-tile first
piece (the Tile scheduler orders triggers by modeled dep-times, and its
hardcoded collective cost model (15us + size/40GBps) overestimates AG
durations ~3x, pushing C-dependent work late in the queue; an earlier-
dependency first piece is the workaround that sticks).

Measured (profiled on HW, run variance +-3%): original baseline 767us
span with Pool 543us busy / Tensor 295us. Final: ~665-680us (mean
~674us) with Pool ~450us / Tensor ~325us. Remaining idle ~260us: AG
pieces are SKEW-dominated (a 2MB piece measures 17-51us; the slowest
core gates each piece; pieces serialize on one CC stream), so layer
boundaries stall ~60-90us waiting for op1's A+B gate. Rejected by
measurement: dense-adjacency aggregation on the TensorEngine for 2
tiles (+56us: PE runs at the HAM-throttled 1.2GHz clock since tensor
work is bursty — matmul pairs ~300ns vs 131ns warm — and the
10MB/layer x-block stream contends on DMA queues); splitting op1 by AG
piece A/B (+60us: 30 extra ~1us-fixed-cost gather instructions);
4-tile merged gathers (+220us: overlap collapse; the stream is also
near the SDMA packet-processing limit); 5 AG pieces (+35us: extra
stream op + dense-group fragmentation beats the finer pipelining);
single_packet=True (runtime crash); scheduler SWDGE-model inflation
(no effect); emitting pair-4 op1 matmuls early on the last layer to
trim the tail (tail grew instead); xs1/oh1 bufs 2->3 (+120us: extra
slack lets the scheduler reorder badly); op2g(0) pulled to 2nd Pool
slot for an earlier first trigger (+15us: the ag(0) head-of-line stall
costs more than the earlier trigger buys); 3-way op1 split by piece
A1/A2 to fill the A1->A2 landing window (+30us: ten extra gather
instructions plus buffer slack outweigh the fill); 3-piece AG with op1
gated on a single 4MB first piece (+15us: the big piece under gather
DMA contention transfers no faster than the pipelined 2MB pair, and
the first trigger needs both pairs' aggregation). The emission order
here is a measured local optimum — perturbations regress.
Next levers if revisited: reduce cross-core AG skew
(stagger-free triggers), a second CC stream, or a gather mechanism
cheaper than ~8.4ns/row.
"""
import sys, types, ctypes, contextlib

import numpy as np


def _install_ntff_hook():
    # antenv.axon_hooks is missing in this image; provide it so
    # bass_utils trace=True can profile via libaxon_pjrt.so.
    if "antenv.axon_hooks" in sys.modules:
        return
    try:
        import antenv  # noqa: F401
    except ImportError:
        return
    mod = types.ModuleType("antenv.axon_hooks")
    state = {"hook": None}
    mod.set_axon_ntff_profile_hook = lambda h: state.__setitem__("hook", h)
    mod.get_axon_ntff_profile_hook = lambda: state["hook"]
    sys.modules["antenv.axon_hooks"] = mod
    try:
        lib = ctypes.CDLL('/opt/axon/libaxon_pjrt.so')
    except OSError:
        return
    if not hasattr(lib, "axon_start_nrt_profile"):
        return
    lib.axon_start_nrt_profile.argtypes = [ctypes.POINTER(ctypes.c_int64), ctypes.c_size_t]
    lib.axon_start_nrt_profile.restype = ctypes.c_int64
    lib.axon_stop_nrt_profile.argtypes = [ctypes.c_char_p]
    lib.axon_stop_nrt_profile.restype = ctypes.c_int64

    @contextlib.contextmanager
    def _hook(output_dir, device_ids):
        import jax
        jax.devices()
        if device_ids:
            ids = (ctypes.c_int64 * len(device_ids))(*device_ids)
            rc = lib.axon_start_nrt_profile(ids, len(device_ids))
        else:
            rc = lib.axon_start_nrt_profile(None, 0)
        if rc != 0:
            raise RuntimeError(f"axon_start_nrt_profile rc={rc}")
        try:
            yield
        finally:
            n = lib.axon_stop_nrt_profile(str(output_dir).encode())
            print(f"profile: {n} file(s) written to {output_dir}", file=sys.stderr)

    state["hook"] = _hook


_install_ntff_hook()

import concourse.bass2jax as _b2j
_orig_cc_hook = _b2j.neuronx_cc_hook
def _dbg_cc_hook(*a, **kw):
    try:
        return _orig_cc_hook(*a, **kw)
    except BaseException:
        import traceback
        traceback.print_exc()
        raise
_b2j.neuronx_cc_hook = _dbg_cc_hook

import concourse.hw_specs as _hw_specs
# calibrate the tile scheduler's SWDGE model to the measured dma_gather
# descriptor-generation rate (~8 ns/idx on HW; set higher so the scheduler
# slots AG triggers after fewer gathers — measured triggers otherwise fire
# ~60us after their deps are ready)
_hw_specs.TRN2Spec.SWDGE_NS_PER_DESCRIPTOR = 8.0

import concourse.bass as bass
import concourse.tile as tile
from concourse import mybir, bacc
from concourse.bass_utils import run_bass_kernel_spmd

F32 = mybir.dt.float32
F32R = mybir.dt.float32r
BF16 = mybir.dt.bfloat16
I32 = mybir.dt.int32
I16 = mybir.dt.int16

N, D, H, O = 10000, 512, 512, 128
C = 8              # cores
NP = 10240         # padded node count
NCORE = NP // C    # 1280 nodes per core
NT = NCORE // 128  # 10 dst tiles per core
GROUPS = [(0, 2), (2, 4), (4, 8), (8, 10)]  # dense/AG pieces by dst tile range
PD = 2             # dst tiles per gather-dedup pair
NPAIR = NT // PD   # 5 dedup pairs per core
AB_ROWS = 4096     # xg rows gating op1 (AG pieces A1+A2); op1 starts at
                   # A2-land, ~25-50us before piece B lands; op2 covers the
                   # rest and its C gate is absorbed by op1's runtime


def _host_prep(x, edge_index):
    src = np.asarray(edge_index[0], dtype=np.int64)
    dst = np.asarray(edge_index[1], dtype=np.int64)
    deg = np.bincount(dst, minlength=NP).astype(np.float64)
    invdeg = (1.0 / np.maximum(deg, 1.0)).astype(np.float32)

    # piece-wise AllGather layout for layers 1,2: an AG piece covering R
    # local rows per core lays its output out as piece_base + c*R + off.
    # Pieces: tiles 0-1, 2-3 (256 rows each), 4-7 (512), 8-9 (256).
    # x0 is stored in the same remapped order so the same gather indices
    # serve all three layers.
    allnodes = np.arange(NP, dtype=np.int64)
    cc, loc = allnodes // NCORE, allnodes % NCORE
    remap = np.where(
        loc < 256, cc * 256 + loc,
        np.where(loc < 512, 2048 + cc * 256 + (loc - 256),
                 np.where(loc < 1024, 4096 + cc * 512 + (loc - 512),
                          8192 + cc * 256 + (loc - 1024)))).astype(np.int64)

    # per (core, 256-dst dedup pair): unique sources split into op1
    # (remap < AB_ROWS) and op2 (remap >= AB_ROWS); deduping over pairs of
    # dst tiles cuts gathered rows ~10% vs per-tile (each subtile then feeds
    # two count matmuls, one per dst tile). oh[slot, sub, dstoff256] = count
    uniq = [[None] * NPAIR for _ in range(C)]
    n1 = np.zeros((C, NPAIR), np.int64)
    n2 = np.zeros((C, NPAIR), np.int64)
    for c in range(C):
        for p in range(NPAIR):
            g0 = (c * NT + p * PD) * 128
            sel = (dst >= g0) & (dst < g0 + PD * 128)
            s_e = src[sel]
            d_e = (dst[sel] - g0).astype(np.int64)
            us = np.unique(s_e)
            m1 = remap[us] < AB_ROWS
            us1, us2 = us[m1], us[~m1]
            uniq[c][p] = (us1, us2, s_e, d_e)
            n1[c, p], n2[c, p] = len(us1), len(us2)

    cnt1m = n1.max(axis=0)
    cnt2m = n2.max(axis=0)
    T1 = np.maximum(np.ceil(cnt1m / 128).astype(np.int64), 1)
    T2 = np.maximum(np.ceil(cnt2m / 128).astype(np.int64), 1)

    # one dma_gather per (pair, op); segment layout [op1_p | op2_p] per pair
    bases1 = np.zeros(NPAIR, np.int64)
    bases2 = np.zeros(NPAIR, np.int64)
    pos = 0
    for p in range(NPAIR):
        bases1[p] = pos
        pos += int(T1[p])
        bases2[p] = pos
        pos += int(T2[p])
    ST = pos

    gidx12 = np.full((C, 128, ST * 8), -1, np.int16)   # xg/x0r rows
    ohv = np.zeros((C, 128, ST, PD * 128), np.float32)  # [slot, sub, dstoff]

    def fill(tbl, vals, col0):
        # value i lands at gather position col0*128 + i; idx table wraps
        # position j at [j % 16, j // 16], replicated over 8 row groups.
        if len(vals) == 0:
            return
        i = np.arange(len(vals))
        pos = col0 * 128 + i
        for r in range(8):
            tbl[16 * r + pos % 16, pos // 16] = vals

    for c in range(C):
        for p in range(NPAIR):
            us1, us2, s_e, d_e = uniq[c][p]
            slot_of = {}
            for us_i, cm, b in ((us1, cnt1m[p], bases1[p]),
                                (us2, cnt2m[p], bases2[p])):
                # pad to the cross-core max with idx 0 (the -1 tail after
                # that is skipped via num_idxs_reg)
                pp = np.zeros(int(cm), np.int64)
                pp[:len(us_i)] = us_i
                fill(gidx12[c], remap[pp].astype(np.int16), int(b))
                for i, s in enumerate(us_i):
                    slot_of[int(s)] = (int(b) + i // 128, i % 128)
            for s_i, d_i in zip(s_e, d_e):
                sub, pslot = slot_of[int(s_i)]
                ohv[c, pslot, sub, d_i] += 1.0

    x_pad = np.zeros((NP, D), np.float32)
    x_pad[:N] = np.asarray(x, dtype=np.float32)
    x0r = np.zeros((NP, D), np.float32)
    x0r[remap] = x_pad

    invdeg_sb = np.empty((C, 128, NT), np.float32)
    for c in range(C):
        invdeg_sb[c] = invdeg[c * NCORE:(c + 1) * NCORE].reshape(NT, 128).T

    xT0 = np.empty((C, 128, 4, NCORE), np.float32)
    for c in range(C):
        xT0[c] = x_pad[c * NCORE:(c + 1) * NCORE].reshape(NCORE, 4, 128).transpose(2, 1, 0)

    import ml_dtypes
    ohv = ohv.astype(ml_dtypes.bfloat16)
    return (x0r, gidx12, ohv, invdeg_sb, xT0,
            T1, T2, bases1, bases2, cnt1m, cnt2m, ST)


def _wsb(w):
    # [K, M] -> SBUF layout [128, K/128, M], bf16
    import ml_dtypes
    w = np.asarray(w, np.float32)
    return np.ascontiguousarray(
        w.reshape(w.shape[0] // 128, 128, w.shape[1]).transpose(1, 0, 2)
    ).astype(ml_dtypes.bfloat16)


def _bsb(b):
    # [M] -> [128, M/128]
    b = np.asarray(b, np.float32)
    return np.ascontiguousarray(b.reshape(b.shape[0] // 128, 128).T)


def _build_program(T1, T2, bases1, bases2, cnt1m, cnt2m, ST):
    NE1MAX, NE2MAX = int(T1.max()), int(T2.max())
    nc = bacc.Bacc(None, target_bir_lowering=False, debug=False, num_devices=C,
                   dynamic_dma_scratch_size=16384)

    x0_d = nc.declare_dram_parameter("x_full0", [NP, D], BF16, isOutput=False)
    gidx12_d = nc.declare_dram_parameter("gidx12", [128, ST * 8], I16, isOutput=False)
    oh_d = nc.declare_dram_parameter("ohv", [128, ST, PD * 128], BF16, isOutput=False)
    invdeg_d = nc.declare_dram_parameter("invdeg", [128, NT], F32, isOutput=False)
    ident_d = nc.declare_dram_parameter("ident", [128, 128], F32, isOutput=False)
    identb_d = nc.declare_dram_parameter("identb", [128, 128], BF16, isOutput=False)
    zeros_d = nc.declare_dram_parameter("zeros", [128, 1], F32, isOutput=False)
    xT0_d = nc.declare_dram_parameter("xT0", [128, 4, NCORE], BF16, isOutput=False)
    w_d = {}
    for l in range(3):
        w_d[f"wl{l}"] = nc.declare_dram_parameter(f"wl{l}", [128, 4, H], BF16, isOutput=False)
        w_d[f"wr{l}"] = nc.declare_dram_parameter(f"wr{l}", [128, 4, H], BF16, isOutput=False)
        w_d[f"b{l}"] = nc.declare_dram_parameter(f"b{l}", [128, 4], F32, isOutput=False)
    wout_d = nc.declare_dram_parameter("wout", [128, 4, O], BF16, isOutput=False)
    bout_d = nc.declare_dram_parameter("bout", [128, 1], F32, isOutput=False)
    out_d = nc.declare_dram_parameter("out", [NCORE, O], F32, isOutput=True)

    xg = [None, nc.dram_tensor("xg1", [NP, D], BF16, addr_space="Shared"),
          nc.dram_tensor("xg2", [NP, D], BF16, addr_space="Shared")]
    xc = [None, nc.dram_tensor("xc1", [NCORE, D], BF16),
          nc.dram_tensor("xc2", [NCORE, D], BF16)]

    with tile.TileContext(nc) as tc:
        with tc.tile_pool(name="const", bufs=1) as constp, \
             tc.tile_pool(name="xT", bufs=2) as xTp, \
             tc.tile_pool(name="aggT", bufs=1) as aggTp, \
             tc.tile_pool(name="xs1", bufs=2) as xs1p, \
             tc.tile_pool(name="xs2", bufs=2) as xs2p, \
             tc.tile_pool(name="oh1", bufs=2) as oh1p, \
             tc.tile_pool(name="oh2", bufs=2) as oh2p, \
             tc.tile_pool(name="agg", bufs=2) as aggp, \
             tc.tile_pool(name="xnm", bufs=2) as xnmp, \
             tc.tile_pool(name="wts", bufs=2) as wp, \
             tc.tile_pool(name="dmy", bufs=3) as dmyp, \
             tc.tile_pool(name="pa", bufs=4, space="PSUM") as pap, \
             tc.tile_pool(name="pt", bufs=1, space="PSUM") as ptp, \
             tc.tile_pool(name="pd", bufs=2, space="PSUM") as pdp:

            # ---- load constants ----
            gidx12_sb = constp.tile([128, ST * 8], I16)
            nc.sync.dma_start(gidx12_sb[:], gidx12_d[:])
            invdeg_sb = constp.tile([128, NT], F32)
            nc.sync.dma_start(invdeg_sb[:], invdeg_d[:])
            ident = constp.tile([128, 128], F32)
            nc.sync.dma_start(ident[:], ident_d[:])
            identb = constp.tile([128, 128], BF16)
            nc.sync.dma_start(identb[:], identb_d[:])
            wsb = {}
            for l in range(3):
                wsb[f"b{l}"] = constp.tile([128, 4], F32, name=f"bsb{l}")
                nc.sync.dma_start(wsb[f"b{l}"][:], w_d[f"b{l}"][:])
            wout_sb = constp.tile([128, 4, O], BF16)
            nc.sync.dma_start(wout_sb[:], wout_d[:])
            bout_sb = constp.tile([128, 1], F32)
            nc.sync.dma_start(bout_sb[:], bout_d[:])

            xT_cur = xTp.tile([128, 4, NCORE], BF16)
            nc.sync.dma_start(xT_cur[:], xT0_d[:])

            gate_in = None
            for l in range(3):
                gidx = gidx12_sb
                aggT = aggTp.tile([128, 4, NCORE], BF16)
                xT_next = xTp.tile([128, 4, NCORE], BF16)
                wlr = wp.tile([128, 8, H], BF16, name="wlr")
                nc.sync.dma_start(wlr[:, 0:4, :], w_d[f"wl{l}"][:])
                nc.sync.dma_start(wlr[:, 4:8, :], w_d[f"wr{l}"][:])
                wl, wr, bb = wlr[:, 0:4, :], wlr[:, 4:8, :], wsb[f"b{l}"]

                pa_of = {}

                xs1_of, xs2_of = {}, {}

                def do_op1g(p):
                    # gather the pair's op1 srcs (AG pieces A+B); the count
                    # matmuls are emitted separately (do_op1m) so Tensor-queue
                    # order can differ from Pool-queue order.
                    ne = int(T1[p])
                    b0 = int(bases1[p])
                    xs = xs1p.tile([128, NE1MAX, D], BF16, name="xs1")
                    nc.vector.memset(xs[:, ne - 1, :], 0.0)
                    src_ap = x0_d[0:AB_ROWS, :] if l == 0 else xg[l][0:AB_ROWS, :]
                    nc.gpsimd.dma_gather(
                        out_ap=xs[:, :ne, :], in_ap=src_ap,
                        idxs_ap=gidx[:, b0 * 8:(b0 + ne) * 8],
                        num_idxs=ne * 128, num_idxs_reg=int(cnt1m[p]),
                        elem_size=D, single_packet=False)
                    oh = oh1p.tile([128, NE1MAX, PD * 128], BF16, name="oh1")
                    nc.sync.dma_start(oh[:, :ne, :], oh_d[:, b0:b0 + ne, :])
                    xs1_of[p] = (xs, oh)

                def do_op1m(p):
                    # count matmuls for the pair's op1 subtiles; opens both
                    # tiles' PSUM accumulations (one matmul per dst tile per
                    # gathered subtile).
                    ne = int(T1[p])
                    xs, oh = xs1_of.pop(p)
                    for j in range(PD):
                        pa = pap.tile([128, D], F32, name="pa")
                        pa_of[p * PD + j] = pa
                        for e in range(ne):
                            nc.tensor.matmul(
                                pa[:], lhsT=oh[:, e, j * 128:(j + 1) * 128],
                                rhs=xs[:, e, :],
                                start=(e == 0), stop=False)

                def do_op2g(p):
                    # gather the pair's op2 srcs (AG piece C)
                    ne = int(T2[p])
                    b0 = int(bases2[p])
                    xs = xs2p.tile([128, NE2MAX, D], BF16, name="xs2")
                    nc.vector.memset(xs[:, ne - 1, :], 0.0)
                    src_ap = x0_d[:] if l == 0 else xg[l][:]
                    nc.gpsimd.dma_gather(
                        out_ap=xs[:, :ne, :], in_ap=src_ap,
                        idxs_ap=gidx[:, b0 * 8:(b0 + ne) * 8],
                        num_idxs=ne * 128, num_idxs_reg=int(cnt2m[p]),
                        elem_size=D, single_packet=False)
                    oh = oh2p.tile([128, NE2MAX, PD * 128], BF16, name="oh2")
                    nc.sync.dma_start(oh[:, :ne, :], oh_d[:, b0:b0 + ne, :])
                    xs2_of[p] = (xs, oh)

                def do_op2m(p):
                    # close both accumulations, scale by 1/deg, transpose
                    # to feat-major.
                    ne = int(T2[p])
                    xs, oh = xs2_of.pop(p)
                    for j in range(PD):
                        t = p * PD + j
                        pa = pa_of.pop(t)
                        for e in range(ne):
                            nc.tensor.matmul(
                                pa[:], lhsT=oh[:, e, j * 128:(j + 1) * 128],
                                rhs=xs[:, e, :],
                                start=False, stop=(e == ne - 1))
                        agg = aggp.tile([128, D], F32, name="agg")
                        nc.scalar.activation(
                            agg[:], pa[:], mybir.ActivationFunctionType.Copy,
                            scale=invdeg_sb[:, t:t + 1])
                        for k in range(4):
                            pt = ptp.tile([128, 128], F32, name="pt")
                            nc.tensor.transpose(pt[:], agg[:, k * 128:(k + 1) * 128], ident[:])
                            nc.vector.tensor_copy(aggT[:, k, t * 128:(t + 1) * 128], pt[:])

                def do_dense_group(goff, gsz):
                    for m in range(4):
                        pd = pdp.tile([128, 512], F32, name="pd")
                        for k in range(4):
                            nc.tensor.matmul(
                                pd[:, :gsz],
                                lhsT=wl[:, k, m * 128:(m + 1) * 128],
                                rhs=aggT[:, k, goff:goff + gsz],
                                start=(k == 0), stop=False)
                        for k in range(4):
                            nc.tensor.matmul(
                                pd[:, :gsz],
                                lhsT=wr[:, k, m * 128:(m + 1) * 128],
                                rhs=xT_cur[:, k, goff:goff + gsz],
                                start=False, stop=(k == 3))
                        nc.scalar.activation(
                            xT_next[:, m, goff:goff + gsz], pd[:, :gsz],
                            mybir.ActivationFunctionType.Relu,
                            bias=bb[:, m:m + 1])
                    if l < 2:
                        for t in range(goff // 128, (goff + gsz) // 128):
                            xnm = xnmp.tile([128, D], BF16, name="xnm")
                            for k in range(4):
                                pt = ptp.tile([128, 128], BF16, name="ptx")
                                nc.tensor.transpose(
                                    pt[:], xT_next[:, k, t * 128:(t + 1) * 128],
                                    identb[:])
                                nc.vector.tensor_copy(xnm[:, k * 128:(k + 1) * 128], pt[:])
                            nc.sync.dma_start(xc[l + 1][t * 128:(t + 1) * 128, :], xnm[:])

                def do_final_group(t0, t1, xT_src):
                    goff, gsz = t0 * 128, (t1 - t0) * 128
                    pd = pdp.tile([128, 512], F32, name="pd")
                    for k in range(4):
                        nc.tensor.matmul(
                            pd[:, :gsz],
                            lhsT=wout_sb[:, k, :],
                            rhs=xT_src[:, k, goff:goff + gsz],
                            start=(k == 0), stop=(k == 3))
                    oT = aggp.tile([128, 512], F32)
                    nc.scalar.activation(
                        oT[:, :gsz], pd[:, :gsz],
                        mybir.ActivationFunctionType.Identity, bias=bout_sb[:, 0:1])
                    for tt in range(gsz // 128):
                        t = t0 + tt
                        pt = ptp.tile([128, 128], F32)
                        nc.tensor.transpose(pt[:], oT[:, tt * 128:(tt + 1) * 128], ident[:])
                        onm = xnmp.tile([128, O], F32)
                        nc.vector.tensor_copy(onm[:], pt[:])
                        nc.sync.dma_start(out_d[t * 128:(t + 1) * 128, :], onm[:])

                # Pool-queue order: each AG trigger is emitted only after the
                # NEXT group's op1 gathers, so a trigger blocked on its dense
                # group can never starve the gather stream (no head-of-line
                # cycle), while still firing ~2 groups earlier than end-of-layer.
                ag_bounds = [(0, 2048), (2048, 4096), (4096, 8192), (8192, 10240)]

                def do_ag(gi):
                    t0, t1 = GROUPS[gi]
                    with tc.high_priority():
                        do_dense_group(t0 * 128, (t1 - t0) * 128)
                        if l < 2:
                            lo, hi = ag_bounds[gi]
                            nc.gpsimd.collective_compute(
                                "AllGather", mybir.AluOpType.bypass,
                                replica_groups=[list(range(C))],
                                ins=[xc[l + 1][t0 * 128:t1 * 128, :]],
                                outs=[xg[l + 1][lo:hi, :]])
                            return None
                        do_final_group(t0, t1, xT_next)
                    return None

                # Pool-queue order: each AG trigger is emitted after the next
                # op1 pair past the gathers its dense group depends on, so a
                # trigger blocked on its dense group never starves the gather
                # stream, while still firing well before end-of-layer.
                # Pool gets gathers as early as their AG gates allow; Tensor
                # gets each dense piece's matmuls BEFORE later pairs' count
                # matmuls, so the AG-trigger input chain is never queued
                # behind bulk count-matmul work. The first AG piece covers
                # just pair 0's tiles, so its trigger needs only op2m(0).
                if l == 0:
                    # layer 0 gathers from local x0 — no AG gates at all, so
                    # interleave op1/op2 per pair and fire each AG piece as
                    # soon as its pair's aggregation closes (first trigger
                    # ~60us earlier than the gate-ordered layout)
                    do_op1g(0)
                    do_op2g(0)
                    do_op1m(0)
                    do_op2m(0)
                    do_ag(0)
                    do_op1g(1)
                    do_op2g(1)
                    do_op1m(1)
                    do_op2m(1)
                    do_ag(1)
                    do_op1g(2)
                    do_op2g(2)
                    do_op1m(2)
                    do_op2m(2)
                    do_op1g(3)
                    do_op2g(3)
                    do_op1m(3)
                    do_op2m(3)
                    do_ag(2)
                    do_op1g(4)
                    do_op2g(4)
                    do_op1m(4)
                    do_op2m(4)
                    do_ag(3)
                else:
                    do_op1g(0)
                    do_op1g(1)
                    do_op1m(0)
                    do_op1g(2)
                    do_op1m(1)
                    do_op1g(3)
                    do_op1g(4)
                    do_op2g(0)
                    do_op2m(0)
                    do_ag(0)
                    do_op2g(1)
                    do_op1m(2)
                    do_op2m(1)
                    do_ag(1)
                    do_op2g(2)
                    do_op1m(3)
                    do_op2m(2)
                    do_op2g(3)
                    do_op1m(4)
                    do_op2m(3)
                    do_ag(2)
                    do_op2g(4)
                    do_op2m(4)
                    do_ag(3)
                xT_cur = xT_next

    nc.compile()
    return nc


def _run(inputs, trace=False):
    x = inputs["x"]
    edge_index = inputs["edge_index"]
    (x0r, gidx12, ohv, invdeg_sb, xT0,
     T1, T2, bases1, bases2, cnt1m, cnt2m, ST) = _host_prep(x, edge_index)
    nc = _build_program(T1, T2, bases1, bases2, cnt1m, cnt2m, ST)

    import ml_dtypes
    shared = {
        "x_full0": x0r.astype(ml_dtypes.bfloat16),
        "wout": _wsb(inputs["w_out"]),
        "bout": np.asarray(inputs["b_out"], np.float32).reshape(128, 1),
        "ident": np.eye(128, dtype=np.float32),
        "identb": np.eye(128, dtype=np.float32).astype(ml_dtypes.bfloat16),
        "zeros": np.zeros((128, 1), np.float32),
    }
    for l in range(3):
        shared[f"wl{l}"] = _wsb(inputs[f"w_l{l}"])
        shared[f"wr{l}"] = _wsb(inputs[f"w_r{l}"])
        shared[f"b{l}"] = _bsb(inputs[f"b_l{l}"])

    in_maps = []
    for c in range(C):
        m = dict(shared)
        m["gidx12"] = np.ascontiguousarray(gidx12[c])
        m["ohv"] = np.ascontiguousarray(ohv[c])
        m["invdeg"] = np.ascontiguousarray(invdeg_sb[c])
        m["xT0"] = np.ascontiguousarray(xT0[c]).astype(ml_dtypes.bfloat16)
        in_maps.append(m)

    res = run_bass_kernel_spmd(nc, in_maps, list(range(C)), trace=trace)
    out = np.concatenate([res.results[c]["out"] for c in range(C)], axis=0)[:N]
    return out.astype(np.float32), res


def kernel(**inputs):
    out, _ = _run(inputs, trace=False)
    return out


def kernel_timed(**inputs):
    out, res = _run(inputs, trace=True)
    return out, res

